# revision 17
# baseline (speedup 1.0000x reference)
"""Trainium2 Bass kernel for KCVLAKeywordVisionCrossAttention.

Strategy (pure data-parallel, 8 cores x 4 batches):
  Host precomputes the tiny keyword-side math (Q = kwq@Wq^T+bq, A = Q_h@Wk_h
  fused per head, scaled by 1/sqrt(hd)) and ships A^T / X^T in fp16 for the
  scores matmul; everything else stays f32 on device:
    scores = A @ X^T          (PE, fp16 in / f32 accum)
    E = exp(scores), P = E/rowsum               (ACT + DVE)
    PX = P @ X (f32), attn^T = Wv^T-blocks @ PX^T, ao^T = Ow^T @ attn^T
    agg = weighted k-mean, delta_row = agg @ (tanh(rs)*Dw)^T + b
    x' = x + delta_bc (PE outer-product broadcast), LayerNorm via bn_stats,
    updated = Identity(r*x' - mu*r) on ACT.
  Host post: transpose kwctx, mask/renormalize attn_probs exactly as reference.
"""

import sys

sys.path.insert(0, "/opt/trn_rl_repo")

import numpy as np

import concourse.bass as bass
import concourse.bacc as bacc
import concourse.tile as tile
from concourse import mybir
from concourse.bass_utils import run_bass_kernel_spmd

D = 512
H = 8
HD = 64
S = 2048
KW = 16
B = 32
NCORES = 8
BLOC = B // NCORES  # 4 batches per core
NST = S // 128  # 16 token tiles
NEC = D // 128  # 4 feature chunks
LN_EPS = 1e-5

F32 = mybir.dt.float32
F16 = mybir.dt.float16
AX = mybir.AxisListType.X
AF = mybir.ActivationFunctionType


def _emit(nc, t):
    """Emit the per-core program. t = dict of dram APs."""
    tc_ctx = tile.TileContext(nc)
    with tc_ctx as tc:
        with (
            tc.tile_pool(name="w", bufs=1) as wp,
            tc.tile_pool(name="xin", bufs=2) as xin,
            tc.tile_pool(name="xt", bufs=2) as xtp,
            tc.tile_pool(name="at", bufs=2) as atp,
            tc.tile_pool(name="eb", bufs=2) as ebp,
            tc.tile_pool(name="work", bufs=2) as wk,
            tc.tile_pool(name="stat", bufs=4) as stp,
            tc.tile_pool(name="ps_scores", bufs=2, space="PSUM") as psS,
            tc.tile_pool(name="ps_small", bufs=3, space="PSUM") as psM,
            tc.tile_pool(name="ps_px", bufs=1, space="PSUM") as psP,
        ):
            # ---- params, loaded once ----
            wvT = wp.tile([128, NEC, D], F32, name="wvT_sb")
            nc.sync.dma_start(out=wvT, in_=t["wvT"].rearrange("(c p) d -> p c d", p=128))
            owT = wp.tile([128, NEC, D], F32, name="owT_sb")
            nc.sync.dma_start(out=owT, in_=t["owT"].rearrange("(c p) d -> p c d", p=128))
            dwT = wp.tile([128, NEC, D], F32, name="dwT_sb")
            nc.sync.dma_start(out=dwT, in_=t["dwTs"].rearrange("(c p) d -> p c d", p=128))
            ident = wp.tile([128, 128], F32, name="ident_sb")
            nc.sync.dma_start(out=ident, in_=t["ident"][:, :])
            sel = wp.tile([128, KW], F32, name="sel_sb")
            nc.sync.dma_start(out=sel, in_=t["sel"][:, :])
            obpT = wp.tile([128, NEC], F32, name="obpT_sb")
            nc.sync.dma_start(out=obpT, in_=t["obpT"][:, :])
            dbs = wp.tile([1, D], F32, name="dbs_sb")
            nc.sync.dma_start(out=dbs, in_=t["dbs"][:, :])
            wrep = wp.tile([128, BLOC, KW], F32, name="wrep_sb")
            nc.sync.dma_start(out=wrep, in_=t["wrep"].rearrange("b p k -> p b k"))
            ones_row = wp.tile([1, 128], F32, name="ones_row")
            nc.vector.memset(ones_row, 1.0)
            one1 = wp.tile([1, 1], F32, name="one1")
            nc.vector.memset(one1, 1.0)
            eps = wp.tile([128, 1], F32, name="eps_sb")
            nc.vector.memset(eps, LN_EPS)

            for b in range(BLOC):
                # ---- inputs for this batch ----
                x_sb = xin.tile([128, NST, D], F32, name="x_sb", tag="x")
                nc.sync.dma_start(
                    out=x_sb, in_=t["x_nat"][b].rearrange("(p s) d -> p s d", p=128)
                )
                xT = xtp.tile([128, NEC, S], F16, name="xT_sb", tag="xT")
                nc.sync.dma_start(
                    out=xT, in_=t["xT_f16"][b].rearrange("p (c s) -> p c s", c=NEC)
                )
                aT = atp.tile([128, NEC, 128], F16, name="aT_sb", tag="aT")
                nc.sync.dma_start(
                    out=aT, in_=t["aT_f16"][b].rearrange("p (c m) -> p c m", c=NEC)
                )

                # ---- scores = A @ X^T in two double-buffered halves ----
                E = ebp.tile([128, S], F32, name="E_sb", tag="E")
                for half in range(2):
                    sc = psS.tile([128, 1024], F32, name="scores", tag="scores")
                    for ec in range(NEC):
                        for j in range(2):
                            nc.tensor.matmul(
                                sc[:, j * 512 : (j + 1) * 512],
                                lhsT=aT[:, ec, :],
                                rhs=xT[:, ec, half * 1024 + j * 512 : half * 1024 + (j + 1) * 512],
                                start=(ec == 0),
                                stop=(ec == NEC - 1),
                            )
                    nc.scalar.activation(
                        out=E[:, half * 1024 : (half + 1) * 1024], in_=sc, func=AF.Exp
                    )
                rs = stp.tile([128, 1], F32, name="rs", tag="rs")
                nc.vector.reduce_sum(rs, E, axis=AX)
                rinv = stp.tile([128, 1], F32, name="rinv", tag="rinv")
                nc.vector.reciprocal(rinv, rs)

                # ---- attn_probs = (Sel*rinv)^T @ E ----
                selp = stp.tile([128, KW], F32, name="selp", tag="selp")
                rinv_b16 = bass.AP(tensor=rinv.tensor, offset=rinv.offset, ap=[rinv.ap[0], [0, KW]])
                nc.vector.tensor_mul(selp, sel, rinv_b16)
                probs_sb = wk.tile([16, S], F32, name="probs_sb", tag="probs")
                for j in range(4):
                    pp = psM.tile([16, 512], F32, name="probs_ps", tag="sm")
                    nc.tensor.matmul(
                        pp, lhsT=selp, rhs=E[:, j * 512 : (j + 1) * 512],
                        start=True, stop=True,
                    )
                    nc.scalar.activation(
                        out=probs_sb[:, j * 512 : (j + 1) * 512], in_=pp, func=AF.Copy
                    )
                nc.sync.dma_start(out=t["probs"][b], in_=probs_sb)

                # ---- E^T via PE transposes ----
                ET = ebp.tile([128, NST, 128], F32, name="ET_sb", tag="ET")
                for st in range(NST):
                    ep = psM.tile([128, 128], F32, name="et_ps", tag="sm")
                    nc.tensor.transpose(ep, E[:, st * 128 : (st + 1) * 128], ident)
                    if st % 2 == 0:
                        nc.vector.tensor_copy(out=ET[:, st, :], in_=ep)
                    else:
                        nc.scalar.activation(out=ET[:, st, :], in_=ep, func=AF.Copy)

                # ---- PX = E^T.T @ X, normalized by rinv on eviction ----
                px = psP.tile([128, D], F32, name="px_ps", tag="px")
                for st in range(NST):
                    nc.tensor.matmul(
                        px, lhsT=ET[:, st, :], rhs=x_sb[:, st, :],
                        start=(st == 0), stop=(st == NST - 1),
                    )
                pxn = wk.tile([128, D], F32, name="pxn_sb", tag="pxn")
                rinv_bD = bass.AP(tensor=rinv.tensor, offset=rinv.offset, ap=[rinv.ap[0], [0, D]])
                nc.vector.tensor_mul(pxn, px, rinv_bD)

                # ---- PX^T ----
                pxT = wk.tile([128, NEC, 128], F32, name="pxT_sb", tag="pxT")
                for ec in range(NEC):
                    tp = psM.tile([128, 128], F32, name="pxt_ps", tag="sm")
                    nc.tensor.transpose(tp, pxn[:, ec * 128 : (ec + 1) * 128], ident)
                    nc.vector.tensor_copy(out=pxT[:, ec, :], in_=tp)

                # ---- attn^T: per head  Wv_h^T-chunks @ PX^T-chunks ----
                attnT = wk.tile([128, NEC, KW], F32, name="attnT_sb", tag="attnT")
                for h in range(H):
                    ah = psM.tile([64, KW], F32, name="attnT_ps", tag="sm")
                    for ec in range(NEC):
                        nc.tensor.matmul(
                            ah,
                            lhsT=wvT[:, ec, h * HD : (h + 1) * HD],
                            rhs=pxT[:, ec, h * KW : (h + 1) * KW],
                            start=(ec == 0), stop=(ec == NEC - 1),
                        )
                    half = 64 * (h % 2)
                    nc.vector.tensor_copy(
                        out=attnT[half : half + 64, h // 2, :], in_=ah
                    )

                # ---- attn_out^T = Ow^T @ attn^T (+ ob') ----
                aoT = wk.tile([128, NEC, KW], F32, name="aoT_sb", tag="aoT")
                for ec in range(NEC):
                    ap_ = psM.tile([128, KW], F32, name="aoT_ps", tag="sm")
                    for dc in range(NEC):
                        nc.tensor.matmul(
                            ap_,
                            lhsT=owT[:, dc, ec * 128 : (ec + 1) * 128],
                            rhs=attnT[:, dc, :],
                            start=(dc == 0), stop=(dc == NEC - 1),
                        )
                    nc.scalar.activation(
                        out=aoT[:, ec, :], in_=ap_, func=AF.Identity,
                        bias=obpT[:, ec : ec + 1], scale=1.0,
                    )
                nc.sync.dma_start(
                    out=t["kwctxT"][b].rearrange("(c p) k -> p c k", p=128), in_=aoT
                )

                # ---- agg^T[e] = sum_k aoT * w~  ----
                aggT = stp.tile([128, NEC], F32, name="aggT", tag="aggT")
                for ec in range(NEC):
                    tmp = stp.tile([128, KW], F32, name="aggtmp", tag="aggtmp")
                    nc.vector.tensor_mul(tmp, aoT[:, ec, :], wrep[:, b, :])
                    nc.vector.reduce_sum(aggT[:, ec : ec + 1], tmp, axis=AX)

                # ---- delta_row = agg @ DwTs + dbs ----
                dr = psM.tile([1, D], F32, name="dr_ps", tag="sm")
                for ec in range(NEC):
                    nc.tensor.matmul(
                        dr, lhsT=aggT[:, ec : ec + 1], rhs=dwT[:, ec, :],
                        start=(ec == 0), stop=False,
                    )
                nc.tensor.matmul(dr, lhsT=one1, rhs=dbs, start=False, stop=True)
                drow = stp.tile([1, D], F32, name="drow_sb", tag="drow")
                nc.scalar.activation(out=drow, in_=dr, func=AF.Copy)

                # ---- broadcast delta over 128 partitions ----
                dbc_ps = psP.tile([128, D], F32, name="dbc_ps", tag="px")
                nc.tensor.matmul(dbc_ps, lhsT=ones_row, rhs=drow, start=True, stop=True)
                dbc = wk.tile([128, D], F32, name="dbc_sb", tag="dbc")
                nc.scalar.activation(out=dbc, in_=dbc_ps, func=AF.Copy)

                # ---- x' = x + dbc (in place), stats, normalize ----
                mv = wk.tile([128, NST, 2], F32, name="mv_sb", tag="mv")
                for st in range(NST):
                    eng = nc.vector if st % 2 == 0 else nc.gpsimd
                    eng.tensor_add(x_sb[:, st, :], x_sb[:, st, :], dbc)
                    stats = stp.tile([128, 6], F32, name="bnst", tag="bnst")
                    nc.vector.bn_stats(out=stats, in_=x_sb[:, st, :])
                    nc.vector.bn_aggr(out=mv[:, st, :], in_=stats)

                rstd = stp.tile([128, NST], F32, name="rstd", tag="rstd")
                nc.scalar.activation(
                    out=rstd, in_=mv[:, :, 1], func=AF.Sqrt, bias=eps, scale=1.0
                )
                nc.vector.reciprocal(rstd, rstd)
                negmur = stp.tile([128, NST], F32, name="negmur", tag="negmur")
                nc.vector.tensor_scalar(
                    negmur, mv[:, :, 0], -1.0, None, op0=mybir.AluOpType.mult
                )
                nc.vector.tensor_mul(negmur, negmur, rstd)

                for st in range(NST):
                    nc.scalar.activation(
                        out=x_sb[:, st, :], in_=x_sb[:, st, :], func=AF.Identity,
                        bias=negmur[:, st : st + 1], scale=rstd[:, st : st + 1],
                    )
                nc.sync.dma_start(
                    out=t["upd"][b].rearrange("(p s) d -> p s d", p=128), in_=x_sb
                )
    return tc_ctx


_CACHE = {}


def _build():
    if "nc" in _CACHE:
        return _CACHE["nc"]
    nc = bacc.Bacc(target_bir_lowering=False)
    t = {}
    t["x_nat"] = nc.dram_tensor("x_nat", [BLOC, S, D], F32, kind="ExternalInput")
    t["xT_f16"] = nc.dram_tensor("xT_f16", [BLOC, 128, NEC * S], F16, kind="ExternalInput")
    t["aT_f16"] = nc.dram_tensor("aT_f16", [BLOC, 128, NEC * 128], F16, kind="ExternalInput")
    t["wrep"] = nc.dram_tensor("wrep", [BLOC, 128, KW], F32, kind="ExternalInput")
    t["sel"] = nc.dram_tensor("sel", [128, KW], F32, kind="ExternalInput")
    t["wvT"] = nc.dram_tensor("wvT", [D, D], F32, kind="ExternalInput")
    t["owT"] = nc.dram_tensor("owT", [D, D], F32, kind="ExternalInput")
    t["dwTs"] = nc.dram_tensor("dwTs", [D, D], F32, kind="ExternalInput")
    t["obpT"] = nc.dram_tensor("obpT", [128, NEC], F32, kind="ExternalInput")
    t["dbs"] = nc.dram_tensor("dbs", [1, D], F32, kind="ExternalInput")
    t["ident"] = nc.dram_tensor("ident", [128, 128], F32, kind="ExternalInput")
    t["upd"] = nc.dram_tensor("upd", [BLOC, S, D], F32, kind="ExternalOutput")
    t["probs"] = nc.dram_tensor("probs", [BLOC, KW, S], F32, kind="ExternalOutput")
    t["kwctxT"] = nc.dram_tensor("kwctxT", [BLOC, D, KW], F32, kind="ExternalOutput")
    _emit(nc, t)
    nc.finalize()
    _CACHE["nc"] = nc
    return nc


def _numpy_reference(vision_tokens, vision_pad_masks, keyword_queries, keyword_mask,
                     in_proj_w, in_proj_b, out_w, out_b, delta_w, delta_b,
                     ln_gamma, ln_beta, residual_scale):
    """Fallback: straight numpy port of the reference (never hit for spec inputs)."""
    x = vision_tokens.astype(np.float32)
    Bn, Sn, _ = x.shape
    Kq = keyword_queries.shape[1]
    q = keyword_queries @ in_proj_w[:D].T + in_proj_b[:D]
    k = x @ in_proj_w[D:2 * D].T + in_proj_b[D:2 * D]
    v = x @ in_proj_w[2 * D:].T + in_proj_b[2 * D:]
    q = q.reshape(Bn, Kq, H, HD).transpose(0, 2, 1, 3)
    k = k.reshape(Bn, Sn, H, HD).transpose(0, 2, 1, 3)
    v = v.reshape(Bn, Sn, H, HD).transpose(0, 2, 1, 3)
    scores = np.einsum("bhkd,bhsd->bhks", q, k) / np.sqrt(np.float32(HD))
    scores = np.where(vision_pad_masks[:, None, None, :], scores, np.float32(-1e9))
    scores = scores - scores.max(-1, keepdims=True)
    e = np.exp(scores)
    aw = e / e.sum(-1, keepdims=True)
    attn = np.einsum("bhks,bhsd->bhkd", aw, v)
    attn = attn.transpose(0, 2, 1, 3).reshape(Bn, Kq, D)
    attn_out = attn @ out_w.T + out_b
    attn_probs = aw.mean(axis=1)
    kw = keyword_mask[:, :, None]
    attn_probs = np.where(kw, attn_probs, 0.0)
    attn_probs = np.where(vision_pad_masks[:, None, :], attn_probs, 0.0)
    denom = np.clip(attn_probs.sum(-1, keepdims=True), 1e-6, None)
    attn_probs = np.where(kw, attn_probs / denom, 0.0)
    keyword_context = np.where(kw, attn_out, 0.0)
    w = keyword_mask.astype(np.float32)[:, :, None]
    agg_denom = np.clip(w.sum(axis=1), 1.0, None)
    aggregated = (keyword_context * w).sum(axis=1) / agg_denom
    delta = aggregated @ delta_w.T + delta_b
    scale = np.tanh(residual_scale)
    xx = x + scale * delta[:, None, :]
    mu = xx.mean(-1, keepdims=True)
    var = ((xx - mu) ** 2).mean(-1, keepdims=True)
    ln = (xx - mu) / np.sqrt(var + LN_EPS) * ln_gamma + ln_beta
    upd = np.where(vision_pad_masks[:, :, None], ln, x)
    active = keyword_mask.any(axis=-1)
    updated = np.where(active[:, None, None], upd, x)
    return updated, keyword_context, attn_probs


def kernel(**inputs):
    vision_tokens = np.ascontiguousarray(np.asarray(inputs["vision_tokens"], np.float32))
    vision_pad_masks = np.asarray(inputs["vision_pad_masks"], bool)
    keyword_queries = np.asarray(inputs["keyword_queries"], np.float32)
    keyword_mask = np.asarray(inputs["keyword_mask"], bool)
    in_proj_w = np.asarray(inputs["in_proj_w"], np.float32)
    in_proj_b = np.asarray(inputs["in_proj_b"], np.float32)
    out_w = np.asarray(inputs["out_w"], np.float32)
    out_b = np.asarray(inputs["out_b"], np.float32)
    delta_w = np.asarray(inputs["delta_w"], np.float32)
    delta_b = np.asarray(inputs["delta_b"], np.float32)
    ln_gamma = np.asarray(inputs["ln_gamma"], np.float32)
    ln_beta = np.asarray(inputs["ln_beta"], np.float32)
    residual_scale = np.float32(np.asarray(inputs["residual_scale"]))

    if not (vision_pad_masks.all() and keyword_mask.all()
            and np.all(ln_gamma == 1.0) and np.all(ln_beta == 0.0)):
        return _numpy_reference(vision_tokens, vision_pad_masks, keyword_queries,
                                keyword_mask, in_proj_w, in_proj_b, out_w, out_b,
                                delta_w, delta_b, ln_gamma, ln_beta, residual_scale)

    # ---- host precompute ----
    Wq, Wk, Wv = in_proj_w[:D], in_proj_w[D:2 * D], in_proj_w[2 * D:]
    bq, bv = in_proj_b[:D], in_proj_b[2 * D:]
    Q = keyword_queries @ Wq.T + bq  # [B,K,D]
    # A[(h,k),e] = sum_d Q[k, h*HD+d] * Wk[h*HD+d, e], scaled 1/sqrt(HD)
    Qh = Q.reshape(B, KW, H, HD).transpose(0, 2, 1, 3)  # [B,H,K,HD]
    Wkh = Wk.reshape(H, HD, D)
    A = np.einsum("bhkd,hde->bhke", Qh, Wkh) / np.sqrt(np.float32(HD))
    A = A.reshape(B, H * KW, D)
    # device token order: column j = t*128 + p  <->  token p*16 + t  (p-major
    # contiguous DMA); feature chunks pre-arranged so DMA is [128, contiguous].
    aT = np.ascontiguousarray(A.transpose(0, 2, 1).astype(np.float16))  # [B,D,128]
    aT16 = np.ascontiguousarray(
        aT.reshape(B, NEC, 128, 128).transpose(0, 2, 1, 3).reshape(B, 128, NEC * 128))
    xT = vision_tokens.transpose(0, 2, 1).astype(np.float16)  # [B,D,S]
    xT = xT.reshape(B, D, 128, 16).transpose(0, 1, 3, 2).reshape(B, D, S)  # col perm
    xT16 = np.ascontiguousarray(
        xT.reshape(B, NEC, 128, S).transpose(0, 2, 1, 3).reshape(B, 128, NEC * S))

    wmask = keyword_mask.astype(np.float32)
    agg_denom = np.clip(wmask.sum(1), 1.0, None)  # [B]
    wrep = np.broadcast_to((wmask / agg_denom[:, None])[:, None, :], (B, 128, KW))
    wrep = np.ascontiguousarray(wrep.astype(np.float32))
    sel = np.zeros((128, KW), np.float32)
    for h in range(H):
        sel[h * KW : (h + 1) * KW][np.arange(KW), np.arange(KW)] = 1.0 / H
    obp = out_w @ bv + out_b  # ob' [D]
    obpT = np.ascontiguousarray(obp.reshape(NEC, 128).T)  # [128, NEC]
    ts = np.tanh(residual_scale)
    dwTs = np.ascontiguousarray((ts * delta_w).T)
    dbs = (ts * delta_b).reshape(1, D)
    wvT = np.ascontiguousarray(Wv.T)
    owT = np.ascontiguousarray(out_w.T)
    ident = np.eye(128, dtype=np.float32)

    nc = _build()
    in_maps = []
    for c in range(NCORES):
        sl = slice(c * BLOC, (c + 1) * BLOC)
        in_maps.append({
            "x_nat": vision_tokens[sl],
            "xT_f16": xT16[sl],
            "aT_f16": aT16[sl],
            "wrep": wrep[sl],
            "sel": sel,
            "wvT": wvT,
            "owT": owT,
            "dwTs": dwTs,
            "obpT": obpT,
            "dbs": dbs,
            "ident": ident,
        })
    res = run_bass_kernel_spmd(nc, in_maps, core_ids=list(range(NCORES)),
                               **_CACHE.get("run_kwargs", {}))
    _CACHE["last_results"] = res

    upd = np.concatenate([r["upd"] for r in res.results], 0)  # [B,S,D]
    probs = np.concatenate([r["probs"] for r in res.results], 0)  # [B,K,S] dev order
    probs = probs.reshape(B, KW, 16, 128).transpose(0, 1, 3, 2).reshape(B, KW, S)
    kwctx = np.concatenate(
        [r["kwctxT"].transpose(0, 2, 1) for r in res.results], 0
    )  # [B,K,D]

    # exact reference mask/renorm semantics on host
    kw = keyword_mask[:, :, None]
    probs = np.where(kw, probs, 0.0)
    probs = np.where(vision_pad_masks[:, None, :], probs, 0.0)
    denom = np.clip(probs.sum(-1, keepdims=True), 1e-6, None)
    probs = np.where(kw, probs / denom, 0.0)
    kwctx = np.where(kw, kwctx, 0.0)
    return upd, kwctx, probs
